# revision 1
# baseline (speedup 1.0000x reference)
"""Trainium2 Bass kernel for nn_Attention_6992206758310.

Dense transformer block: LayerNorm -> QKV -> selective RoPE -> head-last
masked attention (softmax over j) -> out-projection.

Sharding: heads (16) are split 2-per-core across 8 NeuronCores (tensor
parallel). LayerNorm is sharded over sequence rows and the normalized
activations are AllGathered in transposed ([dim, i]) layout. Attention for
the 2 local heads runs fully on-core in sim^T [j, i] layout (softmax over
the partition axis becomes a matmul-accumulated column sum via an appended
ones-column on V). The unnormalized per-head outputs + softmax denominators
are AllToAll-resharded from head-parallel to sequence-parallel, normalized
on the receiving core, and projected through w_out so each core emits its
own 256-row slice of the output. Host concatenates slices (no AllReduce).

Matmuls run as float32r (fp32 storage, reduced-mantissa matmul rounding,
4x the fp32 PE rate).
"""
import numpy as np

N_SEQ = 2048
DIM = 1024
H = 16
DH = 64
NC = 8
HPC = 2           # heads per core
CW = HPC * DH     # 128 local head-dim columns
ISL = N_SEQ // NC # 256 rows of x per core (LN shard / AG block width)
LN_EPS = 1e-6
NEG = -1e30

_CACHE = {}


def _av_segments(off):
    """Column segments of a 1024-wide block, split at PSUM bank (512) bounds."""
    if off < 512:
        return [(off, 512), (512, 1024)]
    return [(off, 1024)]


def _build(debug=False):
    import concourse.bass as bass
    import concourse.bacc as bacc
    import concourse.tile as tile
    import concourse.mybir as mybir

    F32 = mybir.dt.float32
    F32R = mybir.dt.float32r
    AF = mybir.ActivationFunctionType
    ALU = mybir.AluOpType
    AX = mybir.AxisListType

    nc = bacc.Bacc("TRN2", target_bir_lowering=False, debug=False, num_devices=NC)

    x_d = nc.dram_tensor("x_sl", [ISL, DIM], F32, kind="ExternalInput")
    wblk_d = nc.dram_tensor("w_blk", [DIM, 3 * CW], F32R, kind="ExternalInput")
    wout_d = nc.dram_tensor("w_out", [DIM, DIM], F32R, kind="ExternalInput")
    qb_d = nc.dram_tensor("qb", [128, 3], F32, kind="ExternalInput")
    cos_d = nc.dram_tensor("cos2t", [CW, N_SEQ], F32, kind="ExternalInput")
    sin_d = nc.dram_tensor("sin2t", [CW, N_SEQ], F32, kind="ExternalInput")
    pb_d = nc.dram_tensor("pb2d", [128, 16], F32, kind="ExternalInput")
    pb01_d = nc.dram_tensor("pb01", [128, 16], F32, kind="ExternalInput")
    tri_d = nc.dram_tensor("tri2", [128, 256], F32, kind="ExternalInput")
    p128_d = nc.dram_tensor("p128", [128, 128], F32R, kind="ExternalInput")
    esel_d = nc.dram_tensor("e_sel", [16, 1024], F32R, kind="ExternalInput")
    ident_d = nc.dram_tensor("ident", [128, 128], F32, kind="ExternalInput")
    out_d = nc.dram_tensor("out_sl", [ISL, DIM], F32, kind="ExternalOutput")
    if debug:
        dbg = {
            "ag": nc.dram_tensor("dbg_ag", [DIM, ISL], F32, kind="ExternalOutput"),
            "qrope": nc.dram_tensor("dbg_qrope", [CW, N_SEQ], F32, kind="ExternalOutput"),
            "krope": nc.dram_tensor("dbg_krope", [CW, N_SEQ], F32, kind="ExternalOutput"),
            "avall": nc.dram_tensor("dbg_avall", [128, 2080], F32, kind="ExternalOutput"),
            "a2ain": nc.dram_tensor("dbg_a2ain", [NC * 130, ISL], F32, kind="ExternalOutput"),
            "a2aout": nc.dram_tensor("dbg_a2aout", [NC * 130, ISL], F32, kind="ExternalOutput"),
            "avn": nc.dram_tensor("dbg_avn", [128, NC * ISL], F32, kind="ExternalOutput"),
        }

    groups = [list(range(NC))]
    KC = DIM // 128  # 8 contraction chunks

    with tile.TileContext(nc) as tc:
        with tc.tile_pool(name="cst", bufs=1) as cst, \
             tc.tile_pool(name="big", bufs=1) as big, \
             tc.tile_pool(name="wrk", bufs=2) as wrk, \
             tc.tile_pool(name="xt", bufs=16) as xtp, \
             tc.tile_pool(name="dram", bufs=1, space="DRAM") as drp:

            ag_in = drp.tile([DIM, ISL], F32R, tag="ag_in")
            ag_out = drp.tile([NC * DIM, ISL], F32R, tag="ag_out", addr_space="Shared")
            a2a_in = drp.tile([NC * 130, ISL], F32R, tag="a2a_in")
            a2a_out = drp.tile([NC * 130, ISL], F32R, tag="a2a_out")

            # ---------- constants ----------
            cos_t = cst.tile([CW, N_SEQ], F32, tag="cos")
            sin_t = cst.tile([CW, N_SEQ], F32, tag="sin")
            pb_t = cst.tile([128, 16], F32, tag="pb")
            pb01_t = cst.tile([128, 16], F32, tag="pb01")
            tri_t = cst.tile([128, 256], F32, tag="tri")
            p128_t = cst.tile([128, 128], F32R, tag="p128")
            esel_t = cst.tile([16, 1024], F32R, tag="esel")
            ident_t = cst.tile([128, 128], F32, tag="ident")
            qb_t = cst.tile([128, 3], F32, tag="qb")
            zeps = cst.tile([128, 2], F32, tag="zeps")
            nc.vector.memset(zeps[:, 0:1], 0.0)
            nc.vector.memset(zeps[:, 1:2], LN_EPS)
            nc.sync.dma_start(cos_t[:], cos_d.ap())
            nc.sync.dma_start(sin_t[:], sin_d.ap())
            nc.sync.dma_start(pb_t[:], pb_d.ap())
            nc.sync.dma_start(pb01_t[:], pb01_d.ap())
            nc.sync.dma_start(tri_t[:], tri_d.ap())
            nc.sync.dma_start(p128_t[:], p128_d.ap())
            nc.sync.dma_start(esel_t[:], esel_d.ap())
            nc.sync.dma_start(ident_t[:], ident_d.ap())
            nc.sync.dma_start(qb_t[:], qb_d.ap())

            w_t = []
            for kc in range(KC):
                wt = cst.tile([128, 3 * CW], F32R, tag=f"w{kc}")
                nc.sync.dma_start(wt[:], wblk_d.ap()[kc * 128:(kc + 1) * 128, :])
                w_t.append(wt)
            wo_t = []
            for kc in range(KC):
                wt = cst.tile([128, DIM], F32R, tag=f"wo{kc}")
                nc.sync.dma_start(wt[:], wout_d.ap()[kc * 128:(kc + 1) * 128, :])
                wo_t.append(wt)

            # av lhsT per j-chunk: [v_h0(64) | 1 | v_h1(64) | 1] -> 130 cols each
            av_all = big.tile([128, 16 * 130], F32R, tag="av_all")
            av_v = av_all[:].bitcast(F32).rearrange("p (jc c) -> p jc c", c=130)
            nc.vector.memset(av_v[:, :, 64:65], 1.0)
            nc.vector.memset(av_v[:, :, 129:130], 1.0)

            psA = tc.tile_pool(name="psA", bufs=1, space="PSUM")
            ps = psA.__enter__()
            # ---------- phase 1: sharded LayerNorm + transpose ----------
            xnT_sl = []
            for kc in range(KC):
                t = big.tile([128, ISL], F32R, tag=f"xnT{kc}")
                xnT_sl.append(t)
            for half in range(2):
                xt = wrk.tile([128, DIM], F32, tag="ln_x")
                nc.sync.dma_start(xt[:], x_d.ap()[half * 128:(half + 1) * 128, :])
                s1 = wrk.tile([128, 1], F32, tag="ln_s1")
                nc.vector.tensor_reduce(s1[:], xt[:], axis=AX.X, op=ALU.add)
                nmean = wrk.tile([128, 1], F32, tag="ln_nm")
                nc.vector.tensor_scalar_mul(nmean[:], s1[:], -1.0 / DIM)
                sq = wrk.tile([128, DIM], F32, tag="ln_sq")
                ss = wrk.tile([128, 1], F32, tag="ln_ss")
                nc.scalar.activation(sq[:], xt[:], AF.Square, bias=zeps[:, 0:1], accum_out=ss[:])
                em2 = wrk.tile([128, 1], F32, tag="ln_em2")
                nc.vector.tensor_scalar_mul(em2[:], ss[:], 1.0 / DIM)
                mu2 = wrk.tile([128, 1], F32, tag="ln_mu2")
                nc.scalar.activation(mu2[:], nmean[:], AF.Square, bias=zeps[:, 0:1])
                var = wrk.tile([128, 1], F32, tag="ln_var")
                nc.vector.tensor_sub(var[:], em2[:], mu2[:])
                lnv = wrk.tile([128, 1], F32, tag="ln_lnv")
                nc.scalar.activation(lnv[:], var[:], AF.Ln, bias=zeps[:, 1:2])
                rstd = wrk.tile([128, 1], F32, tag="ln_rstd")
                nc.scalar.activation(rstd[:], lnv[:], AF.Exp, bias=zeps[:, 0:1], scale=-0.5)
                nmr = wrk.tile([128, 1], F32, tag="ln_nmr")
                nc.vector.tensor_mul(nmr[:], nmean[:], rstd[:])
                xn = wrk.tile([128, DIM], F32, tag="ln_xn")
                nc.scalar.activation(xn[:], xt[:], AF.Identity,
                                     bias=nmr[:], scale=rstd[:])
                for kc in range(KC):
                    tp = ps.tile([128, 128], F32, tag="tp", bufs=2)
                    nc.tensor.transpose(tp[:], xn[:, kc * 128:(kc + 1) * 128], ident_t[:])
                    nc.scalar.copy(xnT_sl[kc][:, half * 128:(half + 1) * 128], tp[:])
            for kc in range(KC):
                nc.sync.dma_start(ag_in[kc * 128:(kc + 1) * 128, :], xnT_sl[kc][:])

            if debug:
                nc.sync.dma_start(dbg["ag"].ap(), ag_in[:, :].bitcast(F32))
            # ---------- phase 2: AllGather xnT ----------
            nc.gpsimd.collective_compute(
                "AllGather", ALU.bypass, replica_groups=groups,
                ins=[ag_in.opt()], outs=[ag_out.opt()])

            # ---------- phase 3: qkv^T + rope ----------
            qropeT = big.tile([CW, N_SEQ], F32R, tag="qropeT")
            kropeT = big.tile([CW, N_SEQ], F32R, tag="kropeT")
            vT_sb = big.tile([CW, N_SEQ], F32, tag="vT")
            for ib in range(NC):
                ic = slice(ib * ISL, (ib + 1) * ISL)
                xts = []
                for kc in range(KC):
                    t = xtp.tile([128, ISL], F32R, tag="xt")
                    nc.sync.dma_start(
                        t[:], ag_out[ib * DIM + kc * 128: ib * DIM + (kc + 1) * 128, :])
                    xts.append(t)
                ps_q = ps.tile([128, ISL], F32, tag="pq")
                ps_k = ps.tile([128, ISL], F32, tag="pk")
                ps_v = ps.tile([128, ISL], F32, tag="pv")
                for kc in range(KC):
                    st = (kc == 0); sp = (kc == KC - 1)
                    nc.tensor.matmul(ps_q[:], w_t[kc][:, 0:128], xts[kc][:], start=st, stop=sp)
                    nc.tensor.matmul(ps_k[:], w_t[kc][:, 128:256], xts[kc][:], start=st, stop=sp)
                    nc.tensor.matmul(ps_v[:], w_t[kc][:, 256:384], xts[kc][:], start=st, stop=sp)
                qT_sb = wrk.tile([128, ISL], F32R, tag="qT_sb")
                nc.scalar.activation(qT_sb[:], ps_q[:], AF.Identity, bias=qb_t[:, 0:1])
                kT_sb = wrk.tile([128, ISL], F32R, tag="kT_sb")
                nc.vector.tensor_scalar_add(kT_sb[:], ps_k[:], qb_t[:, 1:2])
                ps_qr = ps.tile([128, ISL], F32, tag="pqr")
                nc.tensor.matmul(ps_qr[:], p128_t[:], qT_sb[:], start=True, stop=True)
                ps_kr = ps.tile([128, ISL], F32, tag="pkr")
                nc.tensor.matmul(ps_kr[:], p128_t[:], kT_sb[:], start=True, stop=True)
                for (src_sb, src_r, dst) in ((qT_sb, ps_qr, qropeT), (kT_sb, ps_kr, kropeT)):
                    t1 = wrk.tile([128, ISL], F32, tag="rp1")
                    nc.gpsimd.tensor_mul(t1[:], src_sb[:].bitcast(F32), cos_t[:, ic])
                    t2 = wrk.tile([128, ISL], F32, tag="rp2")
                    nc.vector.tensor_mul(t2[:], src_r[:], sin_t[:, ic])
                    nc.vector.tensor_add(dst[:, ic], t1[:], t2[:])
                nc.scalar.activation(vT_sb[:, ic], ps_v[:], AF.Identity, bias=qb_t[:, 2:3])

            # ---------- phase 4: v transpose into av_all ----------
            for jc in range(16):
                tp = ps.tile([128, 128], F32, tag="tp", bufs=2)
                nc.tensor.transpose(tp[:], vT_sb[:, jc * 128:(jc + 1) * 128], ident_t[:])
                nc.scalar.copy(av_all[:, jc * 130 + 0: jc * 130 + 64], tp[:, 0:64])
                nc.scalar.copy(av_all[:, jc * 130 + 65: jc * 130 + 129], tp[:, 64:128])

            if debug:
                nc.sync.dma_start(dbg["qrope"].ap(), qropeT[:].bitcast(F32))
                nc.sync.dma_start(dbg["krope"].ap(), kropeT[:].bitcast(F32))
                nc.sync.dma_start(dbg["avall"].ap(), av_all[:].bitcast(F32))
            psA.__exit__(None, None, None)
            psB = tc.tile_pool(name="psB", bufs=1, space="PSUM")
            ps = psB.__enter__()
            # ---------- phase 5: attention ----------
            for ib4 in range(2):
                i0 = ib4 * 1024
                jmax = 8 * ib4 + 7
                for h in range(2):
                    hs = slice(h * 64, (h + 1) * 64)
                    av = ps.tile([65, 1024], F32, tag="av", bufs=2)
                    for jc in range(jmax + 1):
                        off = max(0, 128 * jc - i0)
                        segs = _av_segments(off)
                        sim = ps.tile([128, 1024], F32, tag="sim", bufs=2)
                        for (a, b) in segs:
                            nc.tensor.matmul(
                                sim[:, a:b],
                                kropeT[hs, jc * 128:(jc + 1) * 128],
                                qropeT[hs, i0 + a:i0 + b],
                                start=True, stop=True, skip_group_check=True)
                        if 128 * jc >= i0:
                            tsel = 0 if jc == 0 else 128
                            nc.vector.tensor_add(
                                sim[:, off:off + 128], sim[:, off:off + 128],
                                tri_t[:, tsel:tsel + 128])
                        e_t = wrk.tile([128, 1024], F32R, tag="e_t")
                        nc.scalar.activation(e_t[:, off:], sim[:, off:], AF.Exp,
                                             bias=pb_t[:, jc:jc + 1])
                        for (a, b) in segs:
                            last = (ib4 == 1 and jc == jmax and b == 1024)
                            nc.tensor.matmul(
                                av[:, a:b],
                                av_all[:, jc * 130 + 65 * h: jc * 130 + 65 * h + 65],
                                e_t[:, a:b],
                                start=(jc == 0), stop=last,
                                skip_group_check=True)
                    if ib4 == 0:
                        # column i=0 attends to all j: chunks 1..15 add col 0 only
                        e0full = ps.tile([128, 1024], F32, tag="sim", bufs=2)
                        e0ps = e0full[:, 0:16]
                        for jc in range(1, 16):
                            nc.tensor.matmul(
                                e0ps[:, jc:jc + 1],
                                kropeT[hs, jc * 128:(jc + 1) * 128].bitcast(F32),
                                qropeT[hs, 0:1].bitcast(F32),
                                start=(jc == 1), stop=(jc == 15), skip_group_check=True)
                        e0e = wrk.tile([128, 16], F32, tag="e0e")
                        nc.scalar.activation(e0e[:], e0ps[:], AF.Exp, bias=zeps[:, 0:1])
                        e0m = wrk.tile([128, 16], F32, tag="e0m")
                        nc.vector.tensor_mul(e0m[:], e0e[:], pb01_t[:])
                        for jc in range(1, 16):
                            nc.tensor.matmul(
                                av[:, 0:1],
                                av_all[:, jc * 130 + 65 * h: jc * 130 + 65 * h + 65].bitcast(F32),
                                e0m[:, jc:jc + 1],
                                start=False, stop=(jc == 15), skip_group_check=True)
                    avs = wrk.tile([65, 1024], F32R, tag="avs")
                    nc.scalar.copy(avs[:], av[:])
                    for c in range(4):
                        rr = 4 * ib4 + c
                        cs = slice(c * 256, (c + 1) * 256)
                        nc.sync.dma_start(
                            a2a_in[rr * 130 + 64 * h: rr * 130 + 64 * h + 64, :],
                            avs[0:64, cs])
                        nc.sync.dma_start(
                            a2a_in[rr * 130 + 128 + h: rr * 130 + 128 + h + 1, :],
                            avs[64:65, cs])

            if debug:
                nc.sync.dma_start(dbg["a2ain"].ap(), a2a_in[:, :].bitcast(F32))
            # ---------- phase 6: A2A reshard heads -> sequence ----------
            nc.gpsimd.collective_compute(
                "AllToAll", ALU.bypass, replica_groups=groups,
                ins=[a2a_in.opt()], outs=[a2a_out.opt()])

            psB.__exit__(None, None, None)
            psC = tc.tile_pool(name="psC", bufs=1, space="PSUM")
            ps = psC.__enter__()
            # ---------- phase 7: normalize + out-projection ----------
            if debug:
                nc.sync.dma_start(dbg["a2aout"].ap(), a2a_out[:, :].bitcast(F32))
            s16 = wrk.tile([16, ISL], F32, tag="s16")
            for b in range(NC):
                nc.sync.dma_start(
                    s16[2 * b:2 * b + 2, :].bitcast(F32R),
                    a2a_out[b * 130 + 128: b * 130 + 130, :])
            lgs = wrk.tile([16, ISL], F32, tag="lgs")
            nc.scalar.activation(lgs[:], s16[:], AF.Ln, bias=zeps[0:16, 0:1])
            recl = wrk.tile([16, ISL], F32R, tag="recl")
            nc.scalar.activation(recl[:], lgs[:], AF.Exp, bias=zeps[0:16, 0:1], scale=-1.0)
            rcv_all = big.tile([128, NC * ISL], F32R, tag="rcv_all")
            avn_all = big.tile([128, NC * ISL], F32R, tag="avn_all")
            for b in range(NC):
                bs = slice(b * ISL, (b + 1) * ISL)
                nc.sync.dma_start(rcv_all[:, bs], a2a_out[b * 130: b * 130 + 128, :])
                rb_ps = ps.tile([128, ISL], F32, tag="rb", bufs=2)
                nc.tensor.matmul(rb_ps[:], esel_t[:, b * 128:(b + 1) * 128], recl[:],
                                 start=True, stop=True)
                rb_sb = wrk.tile([128, ISL], F32, tag="rb_sb")
                nc.scalar.copy(rb_sb[:], rb_ps[:])
                nc.vector.tensor_mul(avn_all[:, bs], rcv_all[:, bs].bitcast(F32), rb_sb[:])
            if debug:
                nc.sync.dma_start(dbg["avn"].ap(), avn_all[:].bitcast(F32))
            for icx in range(2):
                op0 = ps.tile([128, 512], F32, tag="op", bufs=2)
                op1 = ps.tile([128, 512], F32, tag="op", bufs=2)
                for kb in range(NC):
                    st = (kb == 0); sp = (kb == NC - 1)
                    lhs = avn_all[:, kb * ISL + icx * 128: kb * ISL + (icx + 1) * 128]
                    nc.tensor.matmul(op0[:], lhs, wo_t[kb][:, 0:512], start=st, stop=sp)
                    nc.tensor.matmul(op1[:], lhs, wo_t[kb][:, 512:1024], start=st, stop=sp)
                ob = wrk.tile([128, DIM], F32, tag="ob")
                nc.scalar.copy(ob[:, 0:512], op0[:])
                nc.scalar.copy(ob[:, 512:1024], op1[:])
                nc.sync.dma_start(out_d.ap()[icx * 128:(icx + 1) * 128, :], ob[:])
            psC.__exit__(None, None, None)

    nc.compile()
    return nc


def _host_prep(x, pos_sin, pos_cos, mask, ln_scale, ln_bias, w_qkv, w_out, b_out):
    f32 = np.float32
    scale = np.float32(DIM ** -0.5)
    x = np.asarray(x, f32); pos_sin = np.asarray(pos_sin, f32)
    pos_cos = np.asarray(pos_cos, f32); mask = np.asarray(mask)
    ln_scale = np.asarray(ln_scale, f32); ln_bias = np.asarray(ln_bias, f32)
    w_qkv = np.asarray(w_qkv, f32); w_out = np.asarray(w_out, f32)

    W = w_qkv * ln_scale[:, None]
    qb_full = (ln_bias @ w_qkv).astype(f32)  # [3072]

    cos_full = np.ones((N_SEQ, DH // 2), f32)
    sin_full = np.zeros((N_SEQ, DH // 2), f32)
    cos_full[1:] = pos_cos
    sin_full[1:] = pos_sin
    cos2t = np.ascontiguousarray(np.tile(np.repeat(cos_full, 2, axis=1).T, (2, 1)))
    sin2t = np.ascontiguousarray(np.tile(np.repeat(sin_full, 2, axis=1).T, (2, 1)))

    pb_vec = np.zeros(N_SEQ, f32)
    pb_vec[1:] = np.where(mask, 0.0, NEG).astype(f32)
    pb2d = np.ascontiguousarray(pb_vec.reshape(16, 128).T)
    pb01 = (pb2d == 0).astype(f32)

    idg = np.arange(128)
    triu = (idg[None, :] >= idg[:, None])
    tri_first = np.where(triu | (idg[None, :] == 0), 0.0, NEG).astype(f32)
    tri_rest = np.where(triu, 0.0, NEG).astype(f32)
    tri2 = np.ascontiguousarray(np.concatenate([tri_first, tri_rest], axis=1))

    p128 = np.zeros((128, 128), f32)
    t = np.arange(64)
    p128[2 * t + 1, 2 * t] = -1.0
    p128[2 * t, 2 * t + 1] = 1.0

    e_sel = np.zeros((16, 1024), f32)
    for k in range(16):
        e_sel[k, k * 64:(k + 1) * 64] = 1.0

    ident = np.eye(128, dtype=f32)
    w_out_c = np.ascontiguousarray(w_out)

    in_maps = []
    for r in range(NC):
        hc = slice(CW * r, CW * (r + 1))
        w_blk = np.ascontiguousarray(np.concatenate(
            [W[:, 0:H * DH][:, hc] * scale,
             W[:, H * DH:2 * H * DH][:, hc],
             W[:, 2 * H * DH:][:, hc]], axis=1))
        qb = np.concatenate(
            [qb_full[0:H * DH][hc] * scale,
             qb_full[H * DH:2 * H * DH][hc],
             qb_full[2 * H * DH:][hc]]).astype(f32)
        in_maps.append({
            "x_sl": np.ascontiguousarray(x[r * ISL:(r + 1) * ISL, :]),
            "w_blk": w_blk,
            "w_out": w_out_c,
            "qb": np.ascontiguousarray(qb.reshape(3, CW).T),
            "cos2t": cos2t, "sin2t": sin2t,
            "pb2d": pb2d, "pb01": pb01, "tri2": tri2,
            "p128": p128, "e_sel": e_sel, "ident": ident,
        })
    return in_maps


def _kernel_impl(inputs, trace=False, tmpdir=None):
    from concourse.bass_utils import run_bass_kernel_spmd
    if "nc" not in _CACHE:
        _CACHE["nc"] = _build()
    nc = _CACHE["nc"]
    in_maps = _host_prep(**inputs)
    kwargs = {}
    if trace:
        import sys, types
        try:
            from antenv.axon_hooks import get_axon_ntff_profile_hook  # noqa: F401
        except ImportError:
            from trn_agent_boot.trn_boot import _ntff_profile_via_ctypes
            hook = _ntff_profile_via_ctypes('/opt/axon/libaxon_pjrt.so')
            mod = types.ModuleType('antenv.axon_hooks')
            mod.get_axon_ntff_profile_hook = lambda: hook
            sys.modules['antenv.axon_hooks'] = mod
        kwargs = {"trace": True, "tmpdir": tmpdir}
    res = run_bass_kernel_spmd(nc, in_maps, list(range(NC)), **kwargs)
    out = np.concatenate([res.results[r]["out_sl"] for r in range(NC)], axis=0)
    out = out + np.asarray(inputs["b_out"], np.float32)[None, :]
    return out, res.exec_time_ns


def kernel(**inputs) -> np.ndarray:
    out, _ = _kernel_impl(inputs)
    return out



# revision 4
# speedup vs baseline: 1.0360x; 1.0360x over previous
"""Trainium2 Bass kernel for nn_Attention_6992206758310.

Dense transformer block: LayerNorm -> QKV -> selective RoPE -> head-last
masked attention (softmax over j) -> out-projection.

Sharding: heads (16) are split 2-per-core across 8 NeuronCores (tensor
parallel). LayerNorm is sharded over sequence rows; normalized activations
are AllGathered in transposed ([dim, i]) bf16 layout. Attention for the 2
local heads runs fully on-core in sim^T [j, i] layout (softmax over the
partition axis becomes a matmul-accumulated column sum via an appended
ones-column on V). Per-head outputs are normalized by their softmax
denominators on the head-parallel side, AllToAll-resharded to
sequence-parallel, and projected through w_out so each core emits its own
256-row slice of the output. Host concatenates slices.

All matmuls run in bf16 (fp32 PSUM accumulate); LayerNorm statistics and
softmax bias/mask stay fp32.
"""
import numpy as np

N_SEQ = 2048
DIM = 1024
H = 16
DH = 64
NC = 8
HPC = 2           # heads per core
CW = HPC * DH     # 128 local head-dim columns
ISL = N_SEQ // NC # 256 rows of x per core (LN shard / AG block width)
LN_EPS = 1e-6
NEG = -1e30

_CACHE = {}


def _av_segments(off):
    """Column segments of a 1024-wide block, split at PSUM bank (512) bounds."""
    if off < 512:
        return [(off, 512), (512, 1024)]
    return [(off, 1024)]


def _build(debug=False):
    import concourse.bass as bass
    import concourse.bacc as bacc
    import concourse.tile as tile
    import concourse.mybir as mybir

    F32 = mybir.dt.float32
    BF = mybir.dt.bfloat16
    AF = mybir.ActivationFunctionType
    ALU = mybir.AluOpType
    AX = mybir.AxisListType

    nc = bacc.Bacc("TRN2", target_bir_lowering=False, debug=False, num_devices=NC)

    x_d = nc.dram_tensor("x_sl", [ISL, DIM], F32, kind="ExternalInput")
    wblk_d = nc.dram_tensor("w_blk", [DIM, 3 * CW], BF, kind="ExternalInput")
    wout_d = nc.dram_tensor("w_out", [DIM, DIM], BF, kind="ExternalInput")
    qb_d = nc.dram_tensor("qb", [128, 3], F32, kind="ExternalInput")
    cos_d = nc.dram_tensor("cos2t", [CW, N_SEQ], BF, kind="ExternalInput")
    sin_d = nc.dram_tensor("sin2t", [CW, N_SEQ], BF, kind="ExternalInput")
    pb_d = nc.dram_tensor("pb2d", [128, 16], F32, kind="ExternalInput")
    pb01_d = nc.dram_tensor("pb01", [128, 16], BF, kind="ExternalInput")
    tri_d = nc.dram_tensor("tri2", [128, 256], F32, kind="ExternalInput")
    p128_d = nc.dram_tensor("p128", [128, 128], BF, kind="ExternalInput")
    ident_d = nc.dram_tensor("ident", [128, 128], BF, kind="ExternalInput")
    out_d = nc.dram_tensor("out_sl", [ISL, DIM], F32, kind="ExternalOutput")
    if debug:
        dbg = {
            "ag": nc.dram_tensor("dbg_ag", [DIM, ISL], F32, kind="ExternalOutput"),
            "qrope": nc.dram_tensor("dbg_qrope", [CW, N_SEQ], F32, kind="ExternalOutput"),
            "krope": nc.dram_tensor("dbg_krope", [CW, N_SEQ], F32, kind="ExternalOutput"),
            "avall": nc.dram_tensor("dbg_avall", [128, 16 * 130], F32, kind="ExternalOutput"),
            "a2ain": nc.dram_tensor("dbg_a2ain", [NC * 128, ISL], F32, kind="ExternalOutput"),
        }

    groups = [list(range(NC))]
    KC = DIM // 128  # 8 contraction chunks

    with tile.TileContext(nc) as tc:
        with tc.tile_pool(name="cst", bufs=1) as cst, \
             tc.tile_pool(name="big", bufs=1) as big, \
             tc.tile_pool(name="wrk", bufs=2) as wrk, \
             tc.tile_pool(name="xt", bufs=16) as xtp, \
             tc.tile_pool(name="dram", bufs=1, space="DRAM") as drp:

            ag_in = drp.tile([DIM, ISL], BF, tag="ag_in")
            ag_out = drp.tile([NC * DIM, ISL], BF, tag="ag_out", addr_space="Shared")
            a2a_in = drp.tile([NC * 128, ISL], BF, tag="a2a_in")
            a2a_out = drp.tile([NC * 128, ISL], BF, tag="a2a_out")

            # ---------- phase 0: x + LN-critical constants first ----------
            xh = []
            for half in range(2):
                xt = wrk.tile([128, DIM], F32, tag="ln_x")
                nc.sync.dma_start(xt[:], x_d.ap()[half * 128:(half + 1) * 128, :])
                xh.append(xt)
            ident_t = cst.tile([128, 128], BF, tag="ident")
            nc.sync.dma_start(ident_t[:], ident_d.ap())
            zeps = cst.tile([128, 2], F32, tag="zeps")
            nc.vector.memset(zeps[:, 0:1], 0.0)
            nc.vector.memset(zeps[:, 1:2], LN_EPS)

            psA = tc.tile_pool(name="psA", bufs=1, space="PSUM")
            ps = psA.__enter__()
            # ---------- phase 1: sharded LayerNorm + transpose (bf16 out) ----------
            xnT_sl = []
            for kc in range(KC):
                t = big.tile([128, ISL], BF, tag=f"xnT{kc}")
                xnT_sl.append(t)
            for half in range(2):
                xt = xh[half]
                s1 = wrk.tile([128, 1], F32, tag="ln_s1")
                nc.vector.tensor_reduce(s1[:], xt[:], axis=AX.X, op=ALU.add)
                nmean = wrk.tile([128, 1], F32, tag="ln_nm")
                nc.vector.tensor_scalar_mul(nmean[:], s1[:], -1.0 / DIM)
                sq = wrk.tile([128, DIM], F32, tag="ln_sq")
                ss = wrk.tile([128, 1], F32, tag="ln_ss")
                nc.scalar.activation(sq[:], xt[:], AF.Square, bias=zeps[:, 0:1], accum_out=ss[:])
                em2 = wrk.tile([128, 1], F32, tag="ln_em2")
                nc.vector.tensor_scalar_mul(em2[:], ss[:], 1.0 / DIM)
                mu2 = wrk.tile([128, 1], F32, tag="ln_mu2")
                nc.scalar.activation(mu2[:], nmean[:], AF.Square, bias=zeps[:, 0:1])
                var = wrk.tile([128, 1], F32, tag="ln_var")
                nc.vector.tensor_sub(var[:], em2[:], mu2[:])
                lnv = wrk.tile([128, 1], F32, tag="ln_lnv")
                nc.scalar.activation(lnv[:], var[:], AF.Ln, bias=zeps[:, 1:2])
                rstd = wrk.tile([128, 1], F32, tag="ln_rstd")
                nc.scalar.activation(rstd[:], lnv[:], AF.Exp, bias=zeps[:, 0:1], scale=-0.5)
                nmr = wrk.tile([128, 1], F32, tag="ln_nmr")
                nc.vector.tensor_mul(nmr[:], nmean[:], rstd[:])
                xn = wrk.tile([128, DIM], BF, tag="ln_xn")
                nc.scalar.activation(xn[:], xt[:], AF.Identity,
                                     bias=nmr[:], scale=rstd[:])
                for kc in range(KC):
                    tp = ps.tile([128, 128], BF, tag="tp", bufs=2)
                    nc.tensor.transpose(tp[:], xn[:, kc * 128:(kc + 1) * 128], ident_t[:])
                    nc.scalar.copy(xnT_sl[kc][:, half * 128:(half + 1) * 128], tp[:])
            for kc in range(KC):
                nc.sync.dma_start(ag_in[kc * 128:(kc + 1) * 128, :], xnT_sl[kc][:])

            if debug:
                nc.sync.dma_start(dbg["ag"].ap(), ag_in[:, :])
            # ---------- phase 2: AllGather xnT (bf16) ----------
            nc.gpsimd.collective_compute(
                "AllGather", ALU.bypass, replica_groups=groups,
                ins=[ag_in.opt()], outs=[ag_out.opt()])

            # ---------- remaining constants (after AG trigger enqueued) ----------
            w_t = []
            for kc in range(KC):
                wt = cst.tile([128, 3 * CW], BF, tag=f"w{kc}")
                nc.sync.dma_start(wt[:], wblk_d.ap()[kc * 128:(kc + 1) * 128, :])
                w_t.append(wt)
            cos_t = cst.tile([CW, N_SEQ], BF, tag="cos")
            sin_t = cst.tile([CW, N_SEQ], BF, tag="sin")
            pb_t = cst.tile([128, 16], F32, tag="pb")
            pb01_t = cst.tile([128, 16], BF, tag="pb01")
            tri_t = cst.tile([128, 256], F32, tag="tri")
            p128_t = cst.tile([128, 128], BF, tag="p128")
            qb_t = cst.tile([128, 3], F32, tag="qb")
            nc.sync.dma_start(cos_t[:], cos_d.ap())
            nc.sync.dma_start(sin_t[:], sin_d.ap())
            nc.sync.dma_start(pb_t[:], pb_d.ap())
            nc.sync.dma_start(pb01_t[:], pb01_d.ap())
            nc.sync.dma_start(tri_t[:], tri_d.ap())
            nc.sync.dma_start(p128_t[:], p128_d.ap())
            nc.sync.dma_start(qb_t[:], qb_d.ap())
            wo_t = []
            for kc in range(KC):
                wt = cst.tile([128, DIM], BF, tag=f"wo{kc}")
                nc.sync.dma_start(wt[:], wout_d.ap()[kc * 128:(kc + 1) * 128, :])
                wo_t.append(wt)
            ones64 = cst.tile([1, 64], BF, tag="ones64")
            nc.vector.memset(ones64[:], 1.0)

            # av lhsT per j-chunk: [v_h0(64) | 1 | v_h1(64) | 1] -> 130 cols each
            av_all = big.tile([128, 16 * 130], BF, tag="av_all")
            av_v = av_all[:].rearrange("p (jc c) -> p jc c", c=130)
            nc.vector.memset(av_v[:, :, 64:65], 1.0)
            nc.vector.memset(av_v[:, :, 129:130], 1.0)

            psA.__exit__(None, None, None)
            psB = tc.tile_pool(name="psB", bufs=1, space="PSUM")
            ps = psB.__enter__()
            # ---------- phase 3: qkv^T (weight-stationary halves) + rope ----------
            qropeT = big.tile([CW, N_SEQ], BF, tag="qropeT")
            kropeT = big.tile([CW, N_SEQ], BF, tag="kropeT")
            vT_sb = big.tile([CW, N_SEQ], BF, tag="vT")
            for half in range(2):
                hc = slice(half * 1024, (half + 1) * 1024)
                xts = []
                for kc in range(KC):
                    t = xtp.tile([128, 1024], BF, tag="xt")
                    for rb in range(4):
                        r = half * 4 + rb
                        nc.sync.dma_start(
                            t[:, rb * ISL:(rb + 1) * ISL],
                            ag_out[r * DIM + kc * 128: r * DIM + (kc + 1) * 128, :])
                    xts.append(t)
                ps_q = ps.tile([128, 1024], F32, tag="pp", bufs=3)
                ps_k = ps.tile([128, 1024], F32, tag="pp", bufs=3)
                ps_v = ps.tile([128, 1024], F32, tag="pp", bufs=3)
                for kc in range(KC):
                    st = (kc == 0); sp = (kc == KC - 1)
                    for seg in range(2):
                        cs = slice(seg * 512, (seg + 1) * 512)
                        nc.tensor.matmul(ps_q[:, cs], w_t[kc][:, 0:128], xts[kc][:, cs],
                                         start=st, stop=sp, skip_group_check=True)
                        nc.tensor.matmul(ps_k[:, cs], w_t[kc][:, 128:256], xts[kc][:, cs],
                                         start=st, stop=sp, skip_group_check=True)
                        nc.tensor.matmul(ps_v[:, cs], w_t[kc][:, 256:384], xts[kc][:, cs],
                                         start=st, stop=sp, skip_group_check=True)
                qT_sb = wrk.tile([128, 1024], BF, tag="qT_sb")
                nc.scalar.activation(qT_sb[:], ps_q[:], AF.Identity, bias=qb_t[:, 0:1])
                kT_sb = wrk.tile([128, 1024], BF, tag="kT_sb")
                nc.scalar.activation(kT_sb[:], ps_k[:], AF.Identity, bias=qb_t[:, 1:2])
                nc.scalar.activation(vT_sb[:, hc], ps_v[:], AF.Identity, bias=qb_t[:, 2:3])
                ps_qr = ps.tile([128, 1024], F32, tag="pp", bufs=3)
                ps_kr = ps.tile([128, 1024], F32, tag="pp", bufs=3)
                for seg in range(2):
                    cs = slice(seg * 512, (seg + 1) * 512)
                    nc.tensor.matmul(ps_qr[:, cs], p128_t[:], qT_sb[:, cs],
                                     start=True, stop=True, skip_group_check=True)
                    nc.tensor.matmul(ps_kr[:, cs], p128_t[:], kT_sb[:, cs],
                                     start=True, stop=True, skip_group_check=True)
                for (src_sb, src_r, dst) in ((qT_sb, ps_qr, qropeT), (kT_sb, ps_kr, kropeT)):
                    rr = wrk.tile([128, 1024], BF, tag="rp0")
                    nc.scalar.copy(rr[:], src_r[:])
                    t1 = wrk.tile([128, 1024], BF, tag="rp1")
                    nc.gpsimd.tensor_mul(t1[:], src_sb[:], cos_t[:, hc])
                    t2 = wrk.tile([128, 1024], BF, tag="rp2")
                    nc.vector.tensor_mul(t2[:], rr[:], sin_t[:, hc])
                    nc.vector.tensor_add(dst[:, hc], t1[:], t2[:])

            # ---------- phase 4: v transpose into av_all ----------
            for jc in range(16):
                tp = ps.tile([128, 128], BF, tag="tp", bufs=2)
                nc.tensor.transpose(tp[:], vT_sb[:, jc * 128:(jc + 1) * 128], ident_t[:])
                nc.scalar.copy(av_all[:, jc * 130 + 0: jc * 130 + 64], tp[:, 0:64])
                nc.scalar.copy(av_all[:, jc * 130 + 65: jc * 130 + 129], tp[:, 64:128])

            if debug:
                nc.sync.dma_start(dbg["qrope"].ap(), qropeT[:])
                nc.sync.dma_start(dbg["krope"].ap(), kropeT[:])
                nc.sync.dma_start(dbg["avall"].ap(), av_all[:])
            psB.__exit__(None, None, None)
            psC = tc.tile_pool(name="psC", bufs=1, space="PSUM")
            ps = psC.__enter__()
            # ---------- phase 5: attention + pre-A2A normalize ----------
            for ib4 in range(2):
                i0 = ib4 * 1024
                jmax = 8 * ib4 + 7
                for h in range(2):
                    hs = slice(h * 64, (h + 1) * 64)
                    av = ps.tile([65, 1024], F32, tag="av", bufs=2)
                    for jc in range(jmax + 1):
                        off = max(0, 128 * jc - i0)
                        segs = _av_segments(off)
                        sim = ps.tile([128, 1024], F32, tag="sim", bufs=2)
                        for (a, b) in segs:
                            nc.tensor.matmul(
                                sim[:, a:b],
                                kropeT[hs, jc * 128:(jc + 1) * 128],
                                qropeT[hs, i0 + a:i0 + b],
                                start=True, stop=True, skip_group_check=True)
                        if 128 * jc >= i0:
                            tsel = 0 if jc == 0 else 128
                            nc.vector.tensor_add(
                                sim[:, off:off + 128], sim[:, off:off + 128],
                                tri_t[:, tsel:tsel + 128])
                        e_t = wrk.tile([128, 1024], BF, tag="e_t")
                        nc.scalar.activation(e_t[:, off:], sim[:, off:], AF.Exp,
                                             bias=pb_t[:, jc:jc + 1])
                        for (a, b) in segs:
                            last = (ib4 == 1 and jc == jmax and b == 1024)
                            nc.tensor.matmul(
                                av[:, a:b],
                                av_all[:, jc * 130 + 65 * h: jc * 130 + 65 * h + 65],
                                e_t[:, a:b],
                                start=(jc == 0), stop=last,
                                skip_group_check=True)
                    if ib4 == 0:
                        # column i=0 attends to all j: chunks 1..15 add col 0 only
                        e0full = ps.tile([128, 1024], F32, tag="sim", bufs=2)
                        e0ps = e0full[:, 0:16]
                        for jc in range(1, 16):
                            nc.tensor.matmul(
                                e0ps[:, jc:jc + 1],
                                kropeT[hs, jc * 128:(jc + 1) * 128],
                                qropeT[hs, 0:1],
                                start=(jc == 1), stop=(jc == 15), skip_group_check=True)
                        e0e = wrk.tile([128, 16], BF, tag="e0e")
                        nc.scalar.activation(e0e[:], e0ps[:], AF.Exp, bias=zeps[:, 0:1])
                        e0m = wrk.tile([128, 16], BF, tag="e0m")
                        nc.vector.tensor_mul(e0m[:], e0e[:], pb01_t[:])
                        for jc in range(1, 16):
                            nc.tensor.matmul(
                                av[:, 0:1],
                                av_all[:, jc * 130 + 65 * h: jc * 130 + 65 * h + 65],
                                e0m[:, jc:jc + 1],
                                start=False, stop=(jc == 15), skip_group_check=True)
                    # normalize: avn = av[0:64] / av[64]
                    recl = wrk.tile([1, 1024], F32, tag="recl")
                    nc.vector.reciprocal(recl[:], av[64:65, :])
                    recb = wrk.tile([1, 1024], BF, tag="recb")
                    nc.vector.tensor_scalar_mul(recb[:], recl[:], 1.0)
                    bps = ps.tile([128, 1024], F32, tag="sim", bufs=2)
                    for seg in range(2):
                        cs = slice(seg * 512, (seg + 1) * 512)
                        nc.tensor.matmul(bps[0:64, cs], ones64[:], recb[:, cs],
                                         start=True, stop=True, skip_group_check=True)
                    bsb = wrk.tile([64, 1024], BF, tag="bsb")
                    nc.scalar.copy(bsb[:], bps[0:64, :])
                    avr = wrk.tile([64, 1024], BF, tag="avr")
                    nc.scalar.copy(avr[:], av[0:64, :])
                    avs = wrk.tile([64, 1024], BF, tag="avs")
                    nc.vector.tensor_mul(avs[:], avr[:], bsb[:])
                    for c in range(4):
                        rr = 4 * ib4 + c
                        cs = slice(c * 256, (c + 1) * 256)
                        nc.sync.dma_start(
                            a2a_in[rr * 128 + 64 * h: rr * 128 + 64 * h + 64, :],
                            avs[:, cs])

            if debug:
                nc.sync.dma_start(dbg["a2ain"].ap(), a2a_in[:, :])
            # ---------- phase 6: A2A reshard heads -> sequence (bf16) ----------
            nc.gpsimd.collective_compute(
                "AllToAll", ALU.bypass, replica_groups=groups,
                ins=[a2a_in.opt()], outs=[a2a_out.opt()])

            psC.__exit__(None, None, None)
            psD = tc.tile_pool(name="psD", bufs=1, space="PSUM")
            ps = psD.__enter__()
            # ---------- phase 7: out-projection ----------
            rcv_all = big.tile([128, NC * ISL], BF, tag="rcv_all")
            for b in range(NC):
                bs = slice(b * ISL, (b + 1) * ISL)
                nc.sync.dma_start(rcv_all[:, bs], a2a_out[b * 128: b * 128 + 128, :])
            for icx in range(2):
                op0 = ps.tile([128, 512], F32, tag="op", bufs=2)
                op1 = ps.tile([128, 512], F32, tag="op", bufs=2)
                for kb in range(NC):
                    st = (kb == 0); sp = (kb == NC - 1)
                    lhs = rcv_all[:, kb * ISL + icx * 128: kb * ISL + (icx + 1) * 128]
                    nc.tensor.matmul(op0[:], lhs, wo_t[kb][:, 0:512], start=st, stop=sp)
                    nc.tensor.matmul(op1[:], lhs, wo_t[kb][:, 512:1024], start=st, stop=sp)
                ob = wrk.tile([128, DIM], F32, tag="ob")
                nc.scalar.copy(ob[:, 0:512], op0[:])
                nc.scalar.copy(ob[:, 512:1024], op1[:])
                nc.sync.dma_start(out_d.ap()[icx * 128:(icx + 1) * 128, :], ob[:])
            psD.__exit__(None, None, None)

    nc.compile()
    return nc


def _host_prep(x, pos_sin, pos_cos, mask, ln_scale, ln_bias, w_qkv, w_out, b_out):
    f32 = np.float32
    bf16 = np.dtype('bfloat16') if hasattr(np, 'bfloat16') else None
    import ml_dtypes
    bf16 = ml_dtypes.bfloat16
    scale = np.float32(DIM ** -0.5)
    x = np.asarray(x, f32); pos_sin = np.asarray(pos_sin, f32)
    pos_cos = np.asarray(pos_cos, f32); mask = np.asarray(mask)
    ln_scale = np.asarray(ln_scale, f32); ln_bias = np.asarray(ln_bias, f32)
    w_qkv = np.asarray(w_qkv, f32); w_out = np.asarray(w_out, f32)

    W = w_qkv * ln_scale[:, None]
    qb_full = (ln_bias @ w_qkv).astype(f32)  # [3072]

    cos_full = np.ones((N_SEQ, DH // 2), f32)
    sin_full = np.zeros((N_SEQ, DH // 2), f32)
    cos_full[1:] = pos_cos
    sin_full[1:] = pos_sin
    cos2t = np.ascontiguousarray(np.tile(np.repeat(cos_full, 2, axis=1).T, (2, 1))).astype(bf16)
    sin2t = np.ascontiguousarray(np.tile(np.repeat(sin_full, 2, axis=1).T, (2, 1))).astype(bf16)

    pb_vec = np.zeros(N_SEQ, f32)
    pb_vec[1:] = np.where(mask, 0.0, NEG).astype(f32)
    pb2d = np.ascontiguousarray(pb_vec.reshape(16, 128).T)
    pb01 = np.ascontiguousarray((pb2d == 0)).astype(bf16)

    idg = np.arange(128)
    triu = (idg[None, :] >= idg[:, None])
    tri_first = np.where(triu | (idg[None, :] == 0), 0.0, NEG).astype(f32)
    tri_rest = np.where(triu, 0.0, NEG).astype(f32)
    tri2 = np.ascontiguousarray(np.concatenate([tri_first, tri_rest], axis=1))

    p128 = np.zeros((128, 128), f32)
    t = np.arange(64)
    p128[2 * t + 1, 2 * t] = -1.0
    p128[2 * t, 2 * t + 1] = 1.0
    p128 = p128.astype(bf16)

    ident = np.eye(128, dtype=f32).astype(bf16)
    w_out_c = np.ascontiguousarray(w_out).astype(bf16)

    in_maps = []
    for r in range(NC):
        hc = slice(CW * r, CW * (r + 1))
        w_blk = np.ascontiguousarray(np.concatenate(
            [W[:, 0:H * DH][:, hc] * scale,
             W[:, H * DH:2 * H * DH][:, hc],
             W[:, 2 * H * DH:][:, hc]], axis=1)).astype(bf16)
        qb = np.concatenate(
            [qb_full[0:H * DH][hc] * scale,
             qb_full[H * DH:2 * H * DH][hc],
             qb_full[2 * H * DH:][hc]]).astype(f32)
        in_maps.append({
            "x_sl": np.ascontiguousarray(x[r * ISL:(r + 1) * ISL, :]),
            "w_blk": w_blk,
            "w_out": w_out_c,
            "qb": np.ascontiguousarray(qb.reshape(3, CW).T),
            "cos2t": cos2t, "sin2t": sin2t,
            "pb2d": pb2d, "pb01": pb01, "tri2": tri2,
            "p128": p128, "ident": ident,
        })
    return in_maps


def _kernel_impl(inputs, trace=False, tmpdir=None):
    from concourse.bass_utils import run_bass_kernel_spmd
    if "nc" not in _CACHE:
        _CACHE["nc"] = _build()
    nc = _CACHE["nc"]
    in_maps = _host_prep(**inputs)
    kwargs = {}
    if trace:
        import sys, types
        try:
            from antenv.axon_hooks import get_axon_ntff_profile_hook  # noqa: F401
        except ImportError:
            from trn_agent_boot.trn_boot import _ntff_profile_via_ctypes
            hook = _ntff_profile_via_ctypes('/opt/axon/libaxon_pjrt.so')
            mod = types.ModuleType('antenv.axon_hooks')
            mod.get_axon_ntff_profile_hook = lambda: hook
            sys.modules['antenv.axon_hooks'] = mod
        kwargs = {"trace": True, "tmpdir": tmpdir}
    res = run_bass_kernel_spmd(nc, in_maps, list(range(NC)), **kwargs)
    out = np.concatenate([res.results[r]["out_sl"] for r in range(NC)], axis=0)
    out = out + np.asarray(inputs["b_out"], np.float32)[None, :]
    return out, res.exec_time_ns


def kernel(**inputs) -> np.ndarray:
    out, _ = _kernel_impl(inputs)
    return out


# revision 8
# speedup vs baseline: 1.1548x; 1.1147x over previous
"""Trainium2 Bass kernel for nn_Attention_6992206758310.

Dense transformer block: LayerNorm -> QKV -> selective RoPE -> head-last
masked attention (softmax over j) -> out-projection.

Sharding: heads (16) are split 2-per-core across 8 NeuronCores (tensor
parallel). LayerNorm is REPLICATED on every core (it is cheap and runs
entirely inside the ~66us collective-entry dead window, eliminating the
AllGather a sharded LayerNorm would need). Each core computes QKV + RoPE
for its 2 heads over the full sequence, runs attention in sim^T [j, i]
layout (softmax over the partition axis becomes a matmul-accumulated
column sum via an appended ones-column on V), normalizes by the softmax
denominator on the head-parallel side, AllToAll-reshards to
sequence-parallel, and projects through w_out so each core emits its own
256-row slice of the output. Host concatenates slices.

All matmuls are bf16 (fp32 PSUM accumulate). The scalar engine runs only
Exp (plus one batched Rsqrt + Identity copies) to avoid activation-table
thrash; other elementwise work is on vector/gpsimd. Dummy fp32 matmuls
during the AllToAll keep the PE clock-gate warm for the out-projection.
"""
import numpy as np

N_SEQ = 2048
DIM = 1024
H = 16
DH = 64
NC = 8
HPC = 2           # heads per core
CW = HPC * DH     # 128 local head-dim columns
ISL = N_SEQ // NC # 256 output rows per core
LN_EPS = 1e-6
NEG = -1e30

_CACHE = {}


def _av_segments(off):
    """Column segments of a 1024-wide block, split at PSUM bank (512) bounds."""
    if off < 512:
        return [(off, 512), (512, 1024)]
    return [(off, 1024)]


def _build(debug=False):
    import concourse.bass as bass
    import concourse.bacc as bacc
    import concourse.tile as tile
    import concourse.mybir as mybir

    F32 = mybir.dt.float32
    BF = mybir.dt.bfloat16
    AF = mybir.ActivationFunctionType
    ALU = mybir.AluOpType

    nc = bacc.Bacc("TRN2", target_bir_lowering=False, debug=False, num_devices=NC)

    x_d = nc.dram_tensor("x_bf", [N_SEQ, DIM], BF, kind="ExternalInput")
    wblk_d = nc.dram_tensor("w_blk", [DIM, 3 * CW], BF, kind="ExternalInput")
    wout_d = nc.dram_tensor("w_out", [DIM, DIM], BF, kind="ExternalInput")
    qb_d = nc.dram_tensor("qb", [128, 3], F32, kind="ExternalInput")
    cos_d = nc.dram_tensor("cos2t", [CW, N_SEQ], BF, kind="ExternalInput")
    sin_d = nc.dram_tensor("sin2t", [CW, N_SEQ], BF, kind="ExternalInput")
    pb_d = nc.dram_tensor("pb2d", [128, 16], F32, kind="ExternalInput")
    pb01_d = nc.dram_tensor("pb01", [128, 16], BF, kind="ExternalInput")
    tri_d = nc.dram_tensor("tri2", [128, 256], F32, kind="ExternalInput")
    p128_d = nc.dram_tensor("p128", [128, 128], BF, kind="ExternalInput")
    ident_d = nc.dram_tensor("ident", [128, 128], BF, kind="ExternalInput")
    out_d = nc.dram_tensor("out_sl", [ISL, DIM], F32, kind="ExternalOutput")

    groups = [list(range(NC))]
    KC = DIM // 128  # 8 contraction chunks
    NB = N_SEQ // 128  # 16 sequence blocks

    with tile.TileContext(nc) as tc:
        with tc.tile_pool(name="cst", bufs=1) as cst, \
             tc.tile_pool(name="big", bufs=1) as big, \
             tc.tile_pool(name="wrk", bufs=2) as wrk, \
             tc.tile_pool(name="xb", bufs=16) as xbp, \
             tc.tile_pool(name="et", bufs=4) as etp, \
             tc.tile_pool(name="dram", bufs=1, space="DRAM") as drp:

            a2a_in = drp.tile([NC * 128, ISL], BF, tag="a2a_in")
            a2a_out = drp.tile([NC * 128, ISL], BF, tag="a2a_out")

            # ---------- x blocks + LN-critical constants first ----------
            xblk = []
            for b in range(NB):
                t = xbp.tile([128, DIM], BF, tag="xb")
                nc.sync.dma_start(t[:], x_d.ap()[b * 128:(b + 1) * 128, :])
                xblk.append(t)
            ident_t = cst.tile([128, 128], BF, tag="ident")
            nc.sync.dma_start(ident_t[:], ident_d.ap())
            qb_t = cst.tile([128, 3], F32, tag="qb")
            nc.sync.dma_start(qb_t[:], qb_d.ap())
            w_t = []
            for kc in range(KC):
                wt = cst.tile([128, 3 * CW], BF, tag=f"w{kc}")
                nc.sync.dma_start(wt[:], wblk_d.ap()[kc * 128:(kc + 1) * 128, :])
                w_t.append(wt)
            cos_t = cst.tile([CW, N_SEQ], BF, tag="cos")
            sin_t = cst.tile([CW, N_SEQ], BF, tag="sin")
            pb_t = cst.tile([128, 16], F32, tag="pb")
            pb01_t = cst.tile([128, 16], BF, tag="pb01")
            tri_t = cst.tile([128, 256], F32, tag="tri")
            p128_t = cst.tile([128, 128], BF, tag="p128")
            nc.sync.dma_start(cos_t[:], cos_d.ap())
            nc.sync.dma_start(sin_t[:], sin_d.ap())
            nc.sync.dma_start(pb_t[:], pb_d.ap())
            nc.sync.dma_start(pb01_t[:], pb01_d.ap())
            nc.sync.dma_start(tri_t[:], tri_d.ap())
            nc.sync.dma_start(p128_t[:], p128_d.ap())
            wo_t = []
            for kc in range(KC):
                wt = cst.tile([128, DIM], BF, tag=f"wo{kc}")
                nc.sync.dma_start(wt[:], wout_d.ap()[kc * 128:(kc + 1) * 128, :])
                wo_t.append(wt)

            zeps = cst.tile([128, 2], F32, tag="zeps")
            nc.vector.memset(zeps[:, 0:1], 0.0)
            nc.vector.memset(zeps[:, 1:2], LN_EPS)
            ones64 = cst.tile([1, 64], BF, tag="ones64")
            nc.vector.memset(ones64[:], 1.0)
            # av lhsT per j-chunk: [v_h0(64) | 1 | v_h1(64) | 1] -> 130 cols each
            av_all = big.tile([128, 16 * 130], BF, tag="av_all")
            av_v = av_all[:].rearrange("p (jc c) -> p jc c", c=130)
            nc.vector.memset(av_v[:, :, 64:65], 1.0)
            nc.vector.memset(av_v[:, :, 129:130], 1.0)

            psM = tc.tile_pool(name="psM", bufs=1, space="PSUM")
            ps = psM.__enter__()

            # ---------- phase 1: replicated LayerNorm ----------
            # stats on DVE (bn_stats/bn_aggr), batched Rsqrt on scalar,
            # normalize-apply on gpsimd, transpose copies split scalar/DVE.
            mv_all = wrk.tile([128, 2 * NB], F32, tag="mv")
            st6 = []
            for b in range(NB):
                st = wrk.tile([128, 12], F32, tag="st6", bufs=4)
                nc.vector.bn_stats(st[:, 0:6], xblk[b][:, 0:512])
                nc.vector.bn_stats(st[:, 6:12], xblk[b][:, 512:1024])
                nc.vector.bn_aggr(mv_all[:, 2 * b:2 * b + 2], st[:])
            rstd_all = wrk.tile([128, NB], F32, tag="rstd")
            nmr_all = wrk.tile([128, NB], F32, tag="nmr")
            for g in range(2):
                gs = slice(g * 8, (g + 1) * 8)
                # rstd = 1/sqrt(var + eps)
                sqv = wrk.tile([128, 8], F32, tag="sqv")
                nc.scalar.activation(
                    sqv[:],
                    mv_all[:].rearrange("p (b tw) -> p b tw", tw=2)[:, gs, 1:2],
                    AF.Sqrt, bias=zeps[:, 1:2])
                nc.vector.reciprocal(rstd_all[:, gs], sqv[:])
                # nmr = -mean * rstd
                nc.vector.scalar_tensor_tensor(
                    nmr_all[:, gs],
                    mv_all[:].rearrange("p (b tw) -> p b tw", tw=2)[:, gs, 0:1],
                    -1.0, rstd_all[:, gs], ALU.mult, ALU.mult)
            xnT_sl = []
            for kc in range(KC):
                t = big.tile([128, N_SEQ], BF, tag=f"xnT{kc}")
                xnT_sl.append(t)
            for b in range(NB):
                xn = wrk.tile([128, DIM], BF, tag="ln_xn", bufs=4)
                nc.gpsimd.tensor_scalar(
                    xn[:], xblk[b][:],
                    rstd_all[:, b:b + 1], nmr_all[:, b:b + 1],
                    ALU.mult, ALU.add)
                for grp in range(2):
                    tp = ps.tile([128, 512], BF, tag="tp", bufs=2)
                    for q in range(4):
                        kc = grp * 4 + q
                        nc.tensor.transpose(
                            tp[:, q * 128:(q + 1) * 128],
                            xn[:, kc * 128:(kc + 1) * 128], ident_t[:])
                    for q in range(4):
                        kc = grp * 4 + q
                        dst = xnT_sl[kc][:, b * 128:(b + 1) * 128]
                        src = tp[:, q * 128:(q + 1) * 128]
                        if q % 2 == 0:
                            nc.scalar.copy(dst, src)
                        else:
                            nc.vector.tensor_scalar_mul(dst, src, 1.0)

            # ---------- phase 2: qkv^T (weight-stationary halves) + rope ----------
            qropeT = big.tile([CW, N_SEQ], BF, tag="qropeT")
            kropeT = big.tile([CW, N_SEQ], BF, tag="kropeT")
            vT_sb = big.tile([CW, N_SEQ], BF, tag="vT")
            for half in range(2):
                hc = slice(half * 1024, (half + 1) * 1024)
                ps_q = ps.tile([128, 1024], F32, tag="pp", bufs=3)
                ps_k = ps.tile([128, 1024], F32, tag="pp", bufs=3)
                ps_v = ps.tile([128, 1024], F32, tag="pp", bufs=3)
                for kc in range(KC):
                    st = (kc == 0); sp = (kc == KC - 1)
                    for seg in range(2):
                        cs = slice(seg * 512, (seg + 1) * 512)
                        hs2 = slice(half * 1024 + seg * 512, half * 1024 + (seg + 1) * 512)
                        nc.tensor.matmul(ps_q[:, cs], w_t[kc][:, 0:128], xnT_sl[kc][:, hs2],
                                         start=st, stop=sp, skip_group_check=True)
                        nc.tensor.matmul(ps_k[:, cs], w_t[kc][:, 128:256], xnT_sl[kc][:, hs2],
                                         start=st, stop=sp, skip_group_check=True)
                        nc.tensor.matmul(ps_v[:, cs], w_t[kc][:, 256:384], xnT_sl[kc][:, hs2],
                                         start=st, stop=sp, skip_group_check=True)
                qT_sb = wrk.tile([128, 1024], BF, tag="qT_sb")
                nc.vector.tensor_scalar_add(qT_sb[:], ps_q[:], qb_t[:, 0:1])
                kT_sb = wrk.tile([128, 1024], BF, tag="kT_sb")
                nc.vector.tensor_scalar_add(kT_sb[:], ps_k[:], qb_t[:, 1:2])
                nc.vector.tensor_scalar_add(vT_sb[:, hc], ps_v[:], qb_t[:, 2:3])
                ps_qr = ps.tile([128, 1024], F32, tag="pp", bufs=3)
                ps_kr = ps.tile([128, 1024], F32, tag="pp", bufs=3)
                for seg in range(2):
                    cs = slice(seg * 512, (seg + 1) * 512)
                    nc.tensor.matmul(ps_qr[:, cs], p128_t[:], qT_sb[:, cs],
                                     start=True, stop=True, skip_group_check=True)
                    nc.tensor.matmul(ps_kr[:, cs], p128_t[:], kT_sb[:, cs],
                                     start=True, stop=True, skip_group_check=True)
                for (src_sb, src_r, dst) in ((qT_sb, ps_qr, qropeT), (kT_sb, ps_kr, kropeT)):
                    t1 = wrk.tile([128, 1024], BF, tag="rp1")
                    nc.gpsimd.tensor_mul(t1[:], src_sb[:], cos_t[:, hc])
                    rr = wrk.tile([128, 1024], BF, tag="rp0")
                    nc.vector.tensor_scalar_mul(rr[:], src_r[:], 1.0)
                    t2 = wrk.tile([128, 1024], BF, tag="rp2")
                    nc.vector.tensor_mul(t2[:], rr[:], sin_t[:, hc])
                    nc.vector.tensor_add(dst[:, hc], t1[:], t2[:])

            # ---------- phase 3: v transpose into av_all ----------
            for grp in range(4):
                tp = ps.tile([128, 512], BF, tag="tp", bufs=2)
                for q in range(4):
                    jc = grp * 4 + q
                    nc.tensor.transpose(
                        tp[:, q * 128:(q + 1) * 128],
                        vT_sb[:, jc * 128:(jc + 1) * 128], ident_t[:])
                for q in range(4):
                    jc = grp * 4 + q
                    src = tp[:, q * 128:(q + 1) * 128]
                    eng = nc.scalar.copy if q % 2 == 0 else (
                        lambda d, s: nc.vector.tensor_scalar_mul(d, s, 1.0))
                    eng(av_all[:, jc * 130 + 0: jc * 130 + 64], src[:, 0:64])
                    eng(av_all[:, jc * 130 + 65: jc * 130 + 129], src[:, 64:128])

            psM.__exit__(None, None, None)
            psC = tc.tile_pool(name="psC", bufs=1, space="PSUM")
            ps = psC.__enter__()
            # ---------- phase 4: attention + pre-A2A normalize ----------
            for ib4 in range(2):
                i0 = ib4 * 1024
                jmax = 8 * ib4 + 7
                avh = []
                for h in range(2):
                    av_t = ps.tile([65, 1024], F32, tag="av", bufs=2)
                    avh.append(av_t)
                for jc in range(jmax + 1):
                    off = max(0, 128 * jc - i0)
                    segs = _av_segments(off)
                    for h in range(2):
                        hs = slice(h * 64, (h + 1) * 64)
                        sim = ps.tile([128, 1024], F32, tag="sim", bufs=2)
                        for (a, b) in segs:
                            nc.tensor.matmul(
                                sim[:, a:b],
                                kropeT[hs, jc * 128:(jc + 1) * 128],
                                qropeT[hs, i0 + a:i0 + b],
                                start=True, stop=True, skip_group_check=True)
                        if 128 * jc >= i0:
                            tsel = 0 if jc == 0 else 128
                            nc.vector.tensor_add(
                                sim[:, off:off + 128], sim[:, off:off + 128],
                                tri_t[:, tsel:tsel + 128])
                        e_t = etp.tile([128, 1024], BF, tag="e_t")
                        nc.scalar.activation(e_t[:, off:], sim[:, off:], AF.Exp,
                                             bias=pb_t[:, jc:jc + 1])
                        for (a, b) in segs:
                            last = (ib4 == 1 and jc == jmax and b == 1024)
                            nc.tensor.matmul(
                                avh[h][:, a:b],
                                av_all[:, jc * 130 + 65 * h: jc * 130 + 65 * h + 65],
                                e_t[:, a:b],
                                start=(jc == 0), stop=last,
                                skip_group_check=True)
                for h in range(2):
                    hs = slice(h * 64, (h + 1) * 64)
                    av = avh[h]
                    if ib4 == 0:
                        # column i=0 attends to all j: chunks 1..15 add col 0 only
                        e0full = ps.tile([128, 1024], F32, tag="sim", bufs=2)
                        e0ps = e0full[:, 0:16]
                        for jc in range(1, 16):
                            nc.tensor.matmul(
                                e0ps[:, jc:jc + 1],
                                kropeT[hs, jc * 128:(jc + 1) * 128],
                                qropeT[hs, 0:1],
                                start=(jc == 1), stop=(jc == 15), skip_group_check=True)
                        e0e = wrk.tile([128, 16], BF, tag="e0e")
                        nc.scalar.activation(e0e[:], e0ps[:], AF.Exp, bias=zeps[:, 0:1])
                        e0m = wrk.tile([128, 16], BF, tag="e0m")
                        nc.vector.tensor_mul(e0m[:], e0e[:], pb01_t[:])
                        for jc in range(1, 16):
                            nc.tensor.matmul(
                                av[:, 0:1],
                                av_all[:, jc * 130 + 65 * h: jc * 130 + 65 * h + 65],
                                e0m[:, jc:jc + 1],
                                start=False, stop=(jc == 15), skip_group_check=True)
                    # normalize: avn = av[0:64] / av[64]
                    recl = wrk.tile([1, 1024], F32, tag="recl")
                    nc.vector.reciprocal(recl[:], av[64:65, :])
                    recb = wrk.tile([1, 1024], BF, tag="recb")
                    nc.vector.tensor_scalar_mul(recb[:], recl[:], 1.0)
                    bps = ps.tile([128, 1024], F32, tag="sim", bufs=2)
                    for seg in range(2):
                        cs = slice(seg * 512, (seg + 1) * 512)
                        nc.tensor.matmul(bps[0:64, cs], ones64[:], recb[:, cs],
                                         start=True, stop=True, skip_group_check=True)
                    bsb = wrk.tile([64, 1024], BF, tag="bsb")
                    nc.vector.tensor_scalar_mul(bsb[:], bps[0:64, :], 1.0)
                    avr = wrk.tile([64, 1024], BF, tag="avr")
                    nc.vector.tensor_scalar_mul(avr[:], av[0:64, :], 1.0)
                    avs = wrk.tile([64, 1024], BF, tag="avs")
                    nc.vector.tensor_mul(avs[:], avr[:], bsb[:])
                    for c in range(4):
                        rr2 = 4 * ib4 + c
                        cs = slice(c * 256, (c + 1) * 256)
                        nc.sync.dma_start(
                            a2a_in[rr2 * 128 + 64 * h: rr2 * 128 + 64 * h + 64, :],
                            avs[:, cs])

            # ---------- phase 5: A2A reshard heads -> sequence (bf16) ----------
            nc.gpsimd.collective_compute(
                "AllToAll", ALU.bypass, replica_groups=groups,
                ins=[a2a_in.opt()], outs=[a2a_out.opt()])

            psC.__exit__(None, None, None)
            psD = tc.tile_pool(name="psD", bufs=1, space="PSUM")
            ps = psD.__enter__()
            # ---------- keep the PE clock-gate warm through the A2A ----------
            dmy = ps.tile([128, 256], F32, tag="dmy")
            for _ in range(14):
                nc.tensor.matmul(dmy[:], tri_t[:, 0:128], tri_t[:, 0:256],
                                 start=True, stop=True, skip_group_check=True)
            # ---------- phase 6: out-projection ----------
            rcv_all = big.tile([128, NC * ISL], BF, tag="rcv_all")
            for b in range(NC):
                bs = slice(b * ISL, (b + 1) * ISL)
                nc.sync.dma_start(rcv_all[:, bs], a2a_out[b * 128: b * 128 + 128, :])
            for icx in range(2):
                op0 = ps.tile([128, 512], F32, tag="op", bufs=2)
                op1 = ps.tile([128, 512], F32, tag="op", bufs=2)
                for kb in range(NC):
                    st = (kb == 0); sp = (kb == NC - 1)
                    lhs = rcv_all[:, kb * ISL + icx * 128: kb * ISL + (icx + 1) * 128]
                    nc.tensor.matmul(op0[:], lhs, wo_t[kb][:, 0:512], start=st, stop=sp)
                    nc.tensor.matmul(op1[:], lhs, wo_t[kb][:, 512:1024], start=st, stop=sp)
                ob = wrk.tile([128, DIM], F32, tag="ob")
                nc.vector.tensor_scalar_mul(ob[:, 0:512], op0[:], 1.0)
                nc.vector.tensor_scalar_mul(ob[:, 512:1024], op1[:], 1.0)
                nc.sync.dma_start(out_d.ap()[icx * 128:(icx + 1) * 128, :], ob[:])
            psD.__exit__(None, None, None)

    nc.compile()
    return nc


def _host_prep(x, pos_sin, pos_cos, mask, ln_scale, ln_bias, w_qkv, w_out, b_out):
    f32 = np.float32
    import ml_dtypes
    bf16 = ml_dtypes.bfloat16
    scale = np.float32(DIM ** -0.5)
    x = np.asarray(x, f32); pos_sin = np.asarray(pos_sin, f32)
    pos_cos = np.asarray(pos_cos, f32); mask = np.asarray(mask)
    ln_scale = np.asarray(ln_scale, f32); ln_bias = np.asarray(ln_bias, f32)
    w_qkv = np.asarray(w_qkv, f32); w_out = np.asarray(w_out, f32)

    W = w_qkv * ln_scale[:, None]
    qb_full = (ln_bias @ w_qkv).astype(f32)  # [3072]

    x_bf = np.ascontiguousarray(x).astype(bf16)

    cos_full = np.ones((N_SEQ, DH // 2), f32)
    sin_full = np.zeros((N_SEQ, DH // 2), f32)
    cos_full[1:] = pos_cos
    sin_full[1:] = pos_sin
    cos2t = np.ascontiguousarray(np.tile(np.repeat(cos_full, 2, axis=1).T, (2, 1))).astype(bf16)
    sin2t = np.ascontiguousarray(np.tile(np.repeat(sin_full, 2, axis=1).T, (2, 1))).astype(bf16)

    pb_vec = np.zeros(N_SEQ, f32)
    pb_vec[1:] = np.where(mask, 0.0, NEG).astype(f32)
    pb2d = np.ascontiguousarray(pb_vec.reshape(16, 128).T)
    pb01 = np.ascontiguousarray((pb2d == 0)).astype(bf16)

    idg = np.arange(128)
    triu = (idg[None, :] >= idg[:, None])
    tri_first = np.where(triu | (idg[None, :] == 0), 0.0, NEG).astype(f32)
    tri_rest = np.where(triu, 0.0, NEG).astype(f32)
    tri2 = np.ascontiguousarray(np.concatenate([tri_first, tri_rest], axis=1))

    p128 = np.zeros((128, 128), f32)
    t = np.arange(64)
    p128[2 * t + 1, 2 * t] = -1.0
    p128[2 * t, 2 * t + 1] = 1.0
    p128 = p128.astype(bf16)

    ident = np.eye(128, dtype=f32).astype(bf16)
    w_out_c = np.ascontiguousarray(w_out).astype(bf16)

    in_maps = []
    for r in range(NC):
        hc = slice(CW * r, CW * (r + 1))
        w_blk = np.ascontiguousarray(np.concatenate(
            [W[:, 0:H * DH][:, hc] * scale,
             W[:, H * DH:2 * H * DH][:, hc],
             W[:, 2 * H * DH:][:, hc]], axis=1)).astype(bf16)
        qb = np.concatenate(
            [qb_full[0:H * DH][hc] * scale,
             qb_full[H * DH:2 * H * DH][hc],
             qb_full[2 * H * DH:][hc]]).astype(f32)
        in_maps.append({
            "x_bf": x_bf,
            "w_blk": w_blk,
            "w_out": w_out_c,
            "qb": np.ascontiguousarray(qb.reshape(3, CW).T),
            "cos2t": cos2t, "sin2t": sin2t,
            "pb2d": pb2d, "pb01": pb01, "tri2": tri2,
            "p128": p128, "ident": ident,
        })
    return in_maps


def _kernel_impl(inputs, trace=False, tmpdir=None):
    from concourse.bass_utils import run_bass_kernel_spmd
    if "nc" not in _CACHE:
        _CACHE["nc"] = _build()
    nc = _CACHE["nc"]
    in_maps = _host_prep(**inputs)
    kwargs = {}
    if trace:
        import sys, types
        try:
            from antenv.axon_hooks import get_axon_ntff_profile_hook  # noqa: F401
        except ImportError:
            from trn_agent_boot.trn_boot import _ntff_profile_via_ctypes
            hook = _ntff_profile_via_ctypes('/opt/axon/libaxon_pjrt.so')
            mod = types.ModuleType('antenv.axon_hooks')
            mod.get_axon_ntff_profile_hook = lambda: hook
            sys.modules['antenv.axon_hooks'] = mod
        kwargs = {"trace": True, "tmpdir": tmpdir}
    res = run_bass_kernel_spmd(nc, in_maps, list(range(NC)), **kwargs)
    out = np.concatenate([res.results[r]["out_sl"] for r in range(NC)], axis=0)
    out = out + np.asarray(inputs["b_out"], np.float32)[None, :]
    return out, res.exec_time_ns


def kernel(**inputs) -> np.ndarray:
    out, _ = _kernel_impl(inputs)
    return out


# revision 16
# speedup vs baseline: 1.3963x; 1.2092x over previous
"""Trainium2 Bass kernel for nn_Attention_6992206758310.

Dense transformer block: LayerNorm -> QKV -> selective RoPE -> head-last
masked attention (softmax over j) -> out-projection.

Sharding: heads (16) are split 2-per-core across 8 NeuronCores (tensor
parallel). LayerNorm is REPLICATED on every core (it is cheap and runs
entirely inside the ~66us collective-entry dead window, eliminating the
AllGather a sharded LayerNorm would need). Each core computes QKV + RoPE
for its 2 heads over the full sequence, runs attention in sim^T [j, i]
layout (softmax over the partition axis becomes a matmul-accumulated
column sum via an appended ones-column on V), normalizes by the softmax
denominator on the head-parallel side, AllToAll-reshards to
sequence-parallel, and projects through w_out so each core emits its own
256-row slice of the output. Host concatenates slices.

All matmuls are bf16 (fp32 PSUM accumulate). The scalar engine runs only
Exp (plus one batched Rsqrt + Identity copies) to avoid activation-table
thrash; other elementwise work is on vector/gpsimd. Dummy fp32 matmuls
during the AllToAll keep the PE clock-gate warm for the out-projection.
"""
import numpy as np

N_SEQ = 2048
DIM = 1024
H = 16
DH = 64
NC = 8
HPC = 2           # heads per core
CW = HPC * DH     # 128 local head-dim columns
ISL = N_SEQ // NC # 256 output rows per core
LN_EPS = 1e-6
NEG = -1e30

_CACHE = {}


def _av_segments(off):
    """Column segments of a 1024-wide block, split at PSUM bank (512) bounds."""
    if off < 512:
        return [(off, 512), (512, 1024)]
    return [(off, 1024)]


def _build(debug=False):
    import concourse.bass as bass
    import concourse.bacc as bacc
    import concourse.tile as tile
    import concourse.mybir as mybir

    F32 = mybir.dt.float32
    BF = mybir.dt.bfloat16
    AF = mybir.ActivationFunctionType
    ALU = mybir.AluOpType

    nc = bacc.Bacc("TRN2", target_bir_lowering=False, debug=False, num_devices=NC)

    x_d = nc.dram_tensor("x_bf", [N_SEQ, DIM], BF, kind="ExternalInput")
    wblk_d = nc.dram_tensor("w_blk", [DIM, 3 * CW], BF, kind="ExternalInput")
    wout_d = nc.dram_tensor("w_out", [DIM, DIM], BF, kind="ExternalInput")
    qb_d = nc.dram_tensor("qb", [128, 3], F32, kind="ExternalInput")
    cos_d = nc.dram_tensor("cos2t", [CW, N_SEQ], BF, kind="ExternalInput")
    sin_d = nc.dram_tensor("sin2t", [CW, N_SEQ], BF, kind="ExternalInput")
    pb_d = nc.dram_tensor("pb2d", [128, 16], F32, kind="ExternalInput")
    pb01_d = nc.dram_tensor("pb01", [128, 16], BF, kind="ExternalInput")
    tri_d = nc.dram_tensor("tri2", [128, 256], F32, kind="ExternalInput")
    p128_d = nc.dram_tensor("p128", [128, 128], BF, kind="ExternalInput")
    ident_d = nc.dram_tensor("ident", [128, 128], BF, kind="ExternalInput")
    out_d = nc.dram_tensor("out_sl", [ISL, DIM], F32, kind="ExternalOutput")

    groups = [list(range(NC))]
    KC = DIM // 128  # 8 contraction chunks
    NB = N_SEQ // 128  # 16 sequence blocks

    with tile.TileContext(nc) as tc:
        with tc.tile_pool(name="cst", bufs=1) as cst, \
             tc.tile_pool(name="big", bufs=1) as big, \
             tc.tile_pool(name="wrk", bufs=2) as wrk, \
             tc.tile_pool(name="xb", bufs=1) as xbp, \
             tc.tile_pool(name="et", bufs=4) as etp, \
             tc.tile_pool(name="dram", bufs=1, space="DRAM") as drp:

            a2a_in = drp.tile([NC * 128, ISL], BF, tag="a2a_in")
            a2a_out = drp.tile([NC * 128, ISL], BF, tag="a2a_out")
            wup_in = drp.tile([128, 8], BF, tag="wup_in")
            wup_out = drp.tile([NC * 128, 8], BF, tag="wup_out", addr_space="Shared")

            # ---------- x blocks + LN-critical constants first ----------
            # One 3D DMA for all of x: dst [128, NB, DIM] <- src blocks.
            x_all = xbp.tile([128, NB, DIM], BF, tag="x_all")
            nc.sync.dma_start(
                x_all[:], x_d.ap().rearrange("(b p) d -> p b d", p=128))
            xblk = [x_all[:, b, :] for b in range(NB)]
            ident_t = cst.tile([128, 128], BF, tag="ident")
            nc.sync.dma_start(ident_t[:], ident_d.ap())
            qb_t = cst.tile([128, 3], F32, tag="qb")
            nc.sync.dma_start(qb_t[:], qb_d.ap())
            w_all = cst.tile([128, KC, 3 * CW], BF, tag="w_all")
            nc.sync.dma_start(
                w_all[:], wblk_d.ap().rearrange("(kc p) c -> p kc c", p=128))
            w_t = [w_all[:, kc, :] for kc in range(KC)]
            cos_t = cst.tile([CW, N_SEQ], BF, tag="cos")
            sin_t = cst.tile([CW, N_SEQ], BF, tag="sin")
            pb_t = cst.tile([128, 16], F32, tag="pb")
            pb01_t = cst.tile([128, 16], BF, tag="pb01")
            tri_t = cst.tile([128, 256], F32, tag="tri")
            p128_t = cst.tile([128, 128], BF, tag="p128")
            nc.sync.dma_start(cos_t[:], cos_d.ap())
            nc.sync.dma_start(sin_t[:], sin_d.ap())
            nc.sync.dma_start(pb_t[:], pb_d.ap())
            nc.sync.dma_start(pb01_t[:], pb01_d.ap())
            nc.sync.dma_start(tri_t[:], tri_d.ap())
            nc.sync.dma_start(p128_t[:], p128_d.ap())
            wo_all = cst.tile([128, KC, DIM], BF, tag="wo_all")
            nc.sync.dma_start(
                wo_all[:], wout_d.ap().rearrange("(kc p) c -> p kc c", p=128))
            wo_t = [wo_all[:, kc, :] for kc in range(KC)]

            # tiny warm-up collective: absorbs the collective-entry barrier
            # during LayerNorm and keeps the ncfw stream stepping so the real
            # A2A later starts without a cold-start penalty.
            wup_sb = cst.tile([128, 8], BF, tag="wup_sb")
            nc.vector.memset(wup_sb[:], 0.0)
            nc.sync.dma_start(wup_in[:, :], wup_sb[:])
            nc.gpsimd.collective_compute(
                "AllGather", ALU.bypass, replica_groups=groups,
                ins=[wup_in.opt()], outs=[wup_out.opt()])

            zeps = cst.tile([128, 2], F32, tag="zeps")
            nc.vector.memset(zeps[:, 0:1], 0.0)
            nc.vector.memset(zeps[:, 1:2], LN_EPS)
            ones64 = cst.tile([1, 64], BF, tag="ones64")
            nc.vector.memset(ones64[:], 1.0)
            # av lhsT per j-chunk: [v_h0(64) | 1 | v_h1(64) | 1] -> 130 cols each
            av_all = big.tile([128, 16 * 130], BF, tag="av_all")
            av_v = av_all[:].rearrange("p (jc c) -> p jc c", c=130)
            nc.vector.memset(av_v[:, :, 64:65], 1.0)
            nc.vector.memset(av_v[:, :, 129:130], 1.0)

            psM = tc.tile_pool(name="psM", bufs=1, space="PSUM")
            ps = psM.__enter__()

            # ---------- phase 1: replicated LayerNorm ----------
            # stats on DVE (bn_stats/bn_aggr), batched Rsqrt on scalar,
            # normalize-apply on gpsimd, transpose copies split scalar/DVE.
            mv_all = wrk.tile([128, 2 * NB], F32, tag="mv")
            st6 = []
            for b in range(NB):
                st = wrk.tile([128, 12], F32, tag="st6", bufs=4)
                nc.vector.bn_stats(st[:, 0:6], xblk[b][:, 0:512])
                nc.vector.bn_stats(st[:, 6:12], xblk[b][:, 512:1024])
                nc.vector.bn_aggr(mv_all[:, 2 * b:2 * b + 2], st[:])
            rstd_all = wrk.tile([128, NB], F32, tag="rstd")
            nmr_all = wrk.tile([128, NB], F32, tag="nmr")
            for g in range(2):
                gs = slice(g * 8, (g + 1) * 8)
                # rstd = 1/sqrt(var + eps)
                sqv = wrk.tile([128, 8], F32, tag="sqv")
                nc.scalar.activation(
                    sqv[:],
                    mv_all[:].rearrange("p (b tw) -> p b tw", tw=2)[:, gs, 1:2],
                    AF.Sqrt, bias=zeps[:, 1:2])
                nc.vector.reciprocal(rstd_all[:, gs], sqv[:])
                # nmr = -mean * rstd
                nc.vector.scalar_tensor_tensor(
                    nmr_all[:, gs],
                    mv_all[:].rearrange("p (b tw) -> p b tw", tw=2)[:, gs, 0:1],
                    -1.0, rstd_all[:, gs], ALU.mult, ALU.mult)
            xnT_sl = []
            for kc in range(KC):
                t = big.tile([128, N_SEQ], BF, tag=f"xnT{kc}")
                xnT_sl.append(t)
            for b in range(NB):
                xn = wrk.tile([128, DIM], BF, tag="ln_xn", bufs=4)
                if b % 2 == 0:
                    nc.gpsimd.tensor_scalar(
                        xn[:], xblk[b],
                        rstd_all[:, b:b + 1], nmr_all[:, b:b + 1],
                        ALU.mult, ALU.add)
                else:
                    nc.scalar.activation(
                        xn[:], xblk[b], AF.Identity,
                        bias=nmr_all[:, b:b + 1], scale=rstd_all[:, b:b + 1])
                for grp in range(2):
                    tp = ps.tile([128, 512], BF, tag="tp", bufs=2)
                    for q in range(4):
                        kc = grp * 4 + q
                        nc.tensor.transpose(
                            tp[:, q * 128:(q + 1) * 128],
                            xn[:, kc * 128:(kc + 1) * 128], ident_t[:])
                    for q in range(4):
                        kc = grp * 4 + q
                        dst = xnT_sl[kc][:, b * 128:(b + 1) * 128]
                        src = tp[:, q * 128:(q + 1) * 128]
                        if q % 2 == 0:
                            nc.scalar.copy(dst, src)
                        else:
                            nc.vector.tensor_scalar_mul(dst, src, 1.0)

            # ---------- phase 2: qkv^T (weight-stationary halves) + rope ----------
            qropeT = big.tile([CW, N_SEQ], BF, tag="qropeT")
            kropeT = big.tile([CW, N_SEQ], BF, tag="kropeT")
            vT_sb = big.tile([CW, N_SEQ], BF, tag="vT")
            for half in range(2):
                hc = slice(half * 1024, (half + 1) * 1024)
                ps_q = ps.tile([128, 1024], F32, tag="pp", bufs=3)
                ps_k = ps.tile([128, 1024], F32, tag="pp", bufs=3)
                ps_v = ps.tile([128, 1024], F32, tag="pp", bufs=3)
                for kc in range(KC):
                    st = (kc == 0); sp = (kc == KC - 1)
                    for seg in range(2):
                        cs = slice(seg * 512, (seg + 1) * 512)
                        hs2 = slice(half * 1024 + seg * 512, half * 1024 + (seg + 1) * 512)
                        nc.tensor.matmul(ps_q[:, cs], w_t[kc][:, 0:128], xnT_sl[kc][:, hs2],
                                         start=st, stop=sp, skip_group_check=True)
                        nc.tensor.matmul(ps_k[:, cs], w_t[kc][:, 128:256], xnT_sl[kc][:, hs2],
                                         start=st, stop=sp, skip_group_check=True)
                        nc.tensor.matmul(ps_v[:, cs], w_t[kc][:, 256:384], xnT_sl[kc][:, hs2],
                                         start=st, stop=sp, skip_group_check=True)
                qT_sb = wrk.tile([128, 1024], BF, tag="qT_sb")
                nc.scalar.activation(qT_sb[:], ps_q[:], AF.Identity, bias=qb_t[:, 0:1])
                kT_sb = wrk.tile([128, 1024], BF, tag="kT_sb")
                nc.scalar.activation(kT_sb[:], ps_k[:], AF.Identity, bias=qb_t[:, 1:2])
                nc.scalar.activation(vT_sb[:, hc], ps_v[:], AF.Identity, bias=qb_t[:, 2:3])
                ps_qr = ps.tile([128, 1024], F32, tag="pp", bufs=3)
                ps_kr = ps.tile([128, 1024], F32, tag="pp", bufs=3)
                for seg in range(2):
                    cs = slice(seg * 512, (seg + 1) * 512)
                    nc.tensor.matmul(ps_qr[:, cs], p128_t[:], qT_sb[:, cs],
                                     start=True, stop=True, skip_group_check=True)
                    nc.tensor.matmul(ps_kr[:, cs], p128_t[:], kT_sb[:, cs],
                                     start=True, stop=True, skip_group_check=True)
                for (src_sb, src_r, dst) in ((qT_sb, ps_qr, qropeT), (kT_sb, ps_kr, kropeT)):
                    t1 = wrk.tile([128, 1024], BF, tag="rp1")
                    nc.gpsimd.tensor_mul(t1[:], src_sb[:], cos_t[:, hc])
                    t2 = wrk.tile([128, 1024], BF, tag="rp2")
                    nc.vector.scalar_tensor_tensor(
                        t2[:], src_r[:], 1.0, sin_t[:, hc], ALU.mult, ALU.mult)
                    nc.vector.tensor_add(dst[:, hc], t1[:], t2[:])

            # ---------- phase 3: v transpose into av_all ----------
            for grp in range(4):
                tp = ps.tile([128, 512], BF, tag="tp", bufs=2)
                for q in range(4):
                    jc = grp * 4 + q
                    nc.tensor.transpose(
                        tp[:, q * 128:(q + 1) * 128],
                        vT_sb[:, jc * 128:(jc + 1) * 128], ident_t[:])
                for q in range(4):
                    jc = grp * 4 + q
                    src = tp[:, q * 128:(q + 1) * 128]
                    eng = nc.scalar.copy if q % 2 == 0 else (
                        lambda d, s: nc.vector.tensor_scalar_mul(d, s, 1.0))
                    eng(av_all[:, jc * 130 + 0: jc * 130 + 64], src[:, 0:64])
                    eng(av_all[:, jc * 130 + 65: jc * 130 + 129], src[:, 64:128])

            psM.__exit__(None, None, None)
            psC = tc.tile_pool(name="psC", bufs=1, space="PSUM")
            ps = psC.__enter__()
            # ---------- phase 4: attention + pre-A2A normalize ----------
            for ib4 in range(2):
                i0 = ib4 * 1024
                jmax = 8 * ib4 + 7
                avh = []
                for h in range(2):
                    av_t = ps.tile([65, 1024], F32, tag="av", bufs=2)
                    avh.append(av_t)
                for jc in range(jmax + 1):
                    off = max(0, 128 * jc - i0)
                    segs = _av_segments(off)
                    for h in range(2):
                        hs = slice(h * 64, (h + 1) * 64)
                        sim = ps.tile([128, 1024], F32, tag="sim", bufs=2)
                        for (a, b) in segs:
                            nc.tensor.matmul(
                                sim[:, a:b],
                                kropeT[hs, jc * 128:(jc + 1) * 128],
                                qropeT[hs, i0 + a:i0 + b],
                                start=True, stop=True, skip_group_check=True)
                        if 128 * jc >= i0:
                            tsel = 0 if jc == 0 else 128
                            nc.vector.tensor_add(
                                sim[:, off:off + 128], sim[:, off:off + 128],
                                tri_t[:, tsel:tsel + 128])
                        e_t = etp.tile([128, 1024], BF, tag="e_t")
                        nc.scalar.activation(e_t[:, off:], sim[:, off:], AF.Exp,
                                             bias=pb_t[:, jc:jc + 1])
                        for (a, b) in segs:
                            last = (ib4 == 1 and jc == jmax and b == 1024)
                            nc.tensor.matmul(
                                avh[h][:, a:b],
                                av_all[:, jc * 130 + 65 * h: jc * 130 + 65 * h + 65],
                                e_t[:, a:b],
                                start=(jc == 0), stop=last,
                                skip_group_check=True)
                for h in range(2):
                    hs = slice(h * 64, (h + 1) * 64)
                    av = avh[h]
                    if ib4 == 0:
                        # column i=0 attends to all j: chunks 1..15 add col 0 only
                        e0full = ps.tile([128, 1024], F32, tag="sim", bufs=2)
                        e0ps = e0full[:, 0:16]
                        for jc in range(1, 16):
                            nc.tensor.matmul(
                                e0ps[:, jc:jc + 1],
                                kropeT[hs, jc * 128:(jc + 1) * 128],
                                qropeT[hs, 0:1],
                                start=(jc == 1), stop=(jc == 15), skip_group_check=True)
                        e0e = wrk.tile([128, 16], BF, tag="e0e")
                        nc.scalar.activation(e0e[:], e0ps[:], AF.Exp, bias=zeps[:, 0:1])
                        e0m = wrk.tile([128, 16], BF, tag="e0m")
                        nc.vector.tensor_mul(e0m[:], e0e[:], pb01_t[:])
                        for jc in range(1, 16):
                            nc.tensor.matmul(
                                av[:, 0:1],
                                av_all[:, jc * 130 + 65 * h: jc * 130 + 65 * h + 65],
                                e0m[:, jc:jc + 1],
                                start=False, stop=(jc == 15), skip_group_check=True)
                    # normalize: avn = av[0:64] / av[64]
                    # broadcast the denominator row to 64 partitions via a
                    # ones-matmul, then reciprocal+multiply at full DVE width
                    den = wrk.tile([1, 1024], BF, tag="den")
                    nc.scalar.copy(den[:], av[64:65, :])
                    bps = ps.tile([128, 1024], F32, tag="sim", bufs=2)
                    for seg in range(2):
                        cs = slice(seg * 512, (seg + 1) * 512)
                        nc.tensor.matmul(bps[0:64, cs], ones64[:], den[:, cs],
                                         start=True, stop=True, skip_group_check=True)
                    recb = wrk.tile([64, 1024], F32, tag="recb")
                    nc.vector.reciprocal_approx_fast(recb[:], bps[0:64, :])
                    avs = wrk.tile([64, 1024], BF, tag="avs")
                    nc.vector.tensor_mul(avs[:], av[0:64, :], recb[:])
                    nc.sync.dma_start(
                        a2a_in[:].rearrange("(blk p) i -> p blk i", p=128)
                               [64 * h:64 * h + 64, 4 * ib4:4 * ib4 + 4, :],
                        avs[:].rearrange("p (blk i) -> p blk i", blk=4))

            # ---------- phase 5: A2A reshard heads -> sequence (bf16) ----------
            nc.gpsimd.collective_compute(
                "AllToAll", ALU.bypass, replica_groups=groups,
                ins=[a2a_in.opt()], outs=[a2a_out.opt()])

            psC.__exit__(None, None, None)
            psD = tc.tile_pool(name="psD", bufs=1, space="PSUM")
            ps = psD.__enter__()
            # ---------- keep the PE clock-gate warm through the A2A ----------
            dmy = ps.tile([128, 256], F32, tag="dmy")
            for _ in range(24):
                nc.tensor.matmul(dmy[:], tri_t[:, 0:128], tri_t[:, 0:256],
                                 start=True, stop=True, skip_group_check=True)
            # ---------- phase 6: out-projection ----------
            rcv_all = big.tile([128, NC * ISL], BF, tag="rcv_all")
            nc.sync.dma_start(
                rcv_all[:].rearrange("p (blk i) -> p blk i", blk=NC),
                a2a_out[:].rearrange("(blk p) i -> p blk i", p=128))
            for icx in range(2):
                op0 = ps.tile([128, 512], F32, tag="op", bufs=2)
                op1 = ps.tile([128, 512], F32, tag="op", bufs=2)
                for kb in range(NC):
                    st = (kb == 0); sp = (kb == NC - 1)
                    lhs = rcv_all[:, kb * ISL + icx * 128: kb * ISL + (icx + 1) * 128]
                    nc.tensor.matmul(op0[:], lhs, wo_t[kb][:, 0:512], start=st, stop=sp)
                    nc.tensor.matmul(op1[:], lhs, wo_t[kb][:, 512:1024], start=st, stop=sp)
                ob = wrk.tile([128, DIM], F32, tag="ob")
                nc.vector.tensor_scalar_mul(ob[:, 0:512], op0[:], 1.0)
                nc.vector.tensor_scalar_mul(ob[:, 512:1024], op1[:], 1.0)
                nc.sync.dma_start(out_d.ap()[icx * 128:(icx + 1) * 128, :], ob[:])
            psD.__exit__(None, None, None)

    nc.compile()
    return nc


def _host_prep(x, pos_sin, pos_cos, mask, ln_scale, ln_bias, w_qkv, w_out, b_out):
    f32 = np.float32
    import ml_dtypes
    bf16 = ml_dtypes.bfloat16
    scale = np.float32(DIM ** -0.5)
    x = np.asarray(x, f32); pos_sin = np.asarray(pos_sin, f32)
    pos_cos = np.asarray(pos_cos, f32); mask = np.asarray(mask)
    ln_scale = np.asarray(ln_scale, f32); ln_bias = np.asarray(ln_bias, f32)
    w_qkv = np.asarray(w_qkv, f32); w_out = np.asarray(w_out, f32)

    W = w_qkv * ln_scale[:, None]
    qb_full = (ln_bias @ w_qkv).astype(f32)  # [3072]

    x_bf = np.ascontiguousarray(x).astype(bf16)

    cos_full = np.ones((N_SEQ, DH // 2), f32)
    sin_full = np.zeros((N_SEQ, DH // 2), f32)
    cos_full[1:] = pos_cos
    sin_full[1:] = pos_sin
    cos2t = np.ascontiguousarray(np.tile(np.repeat(cos_full, 2, axis=1).T, (2, 1))).astype(bf16)
    sin2t = np.ascontiguousarray(np.tile(np.repeat(sin_full, 2, axis=1).T, (2, 1))).astype(bf16)

    pb_vec = np.zeros(N_SEQ, f32)
    pb_vec[1:] = np.where(mask, 0.0, NEG).astype(f32)
    pb2d = np.ascontiguousarray(pb_vec.reshape(16, 128).T)
    pb01 = np.ascontiguousarray((pb2d == 0)).astype(bf16)

    idg = np.arange(128)
    triu = (idg[None, :] >= idg[:, None])
    tri_first = np.where(triu | (idg[None, :] == 0), 0.0, NEG).astype(f32)
    tri_rest = np.where(triu, 0.0, NEG).astype(f32)
    tri2 = np.ascontiguousarray(np.concatenate([tri_first, tri_rest], axis=1))

    p128 = np.zeros((128, 128), f32)
    t = np.arange(64)
    p128[2 * t + 1, 2 * t] = -1.0
    p128[2 * t, 2 * t + 1] = 1.0
    p128 = p128.astype(bf16)

    ident = np.eye(128, dtype=f32).astype(bf16)
    w_out_c = np.ascontiguousarray(w_out).astype(bf16)

    in_maps = []
    for r in range(NC):
        hc = slice(CW * r, CW * (r + 1))
        w_blk = np.ascontiguousarray(np.concatenate(
            [W[:, 0:H * DH][:, hc] * scale,
             W[:, H * DH:2 * H * DH][:, hc],
             W[:, 2 * H * DH:][:, hc]], axis=1)).astype(bf16)
        qb = np.concatenate(
            [qb_full[0:H * DH][hc] * scale,
             qb_full[H * DH:2 * H * DH][hc],
             qb_full[2 * H * DH:][hc]]).astype(f32)
        in_maps.append({
            "x_bf": x_bf,
            "w_blk": w_blk,
            "w_out": w_out_c,
            "qb": np.ascontiguousarray(qb.reshape(3, CW).T),
            "cos2t": cos2t, "sin2t": sin2t,
            "pb2d": pb2d, "pb01": pb01, "tri2": tri2,
            "p128": p128, "ident": ident,
        })
    return in_maps


def _kernel_impl(inputs, trace=False, tmpdir=None):
    from concourse.bass_utils import run_bass_kernel_spmd
    if "nc" not in _CACHE:
        _CACHE["nc"] = _build()
    nc = _CACHE["nc"]
    in_maps = _host_prep(**inputs)
    kwargs = {}
    if trace:
        import sys, types
        try:
            from antenv.axon_hooks import get_axon_ntff_profile_hook  # noqa: F401
        except ImportError:
            from trn_agent_boot.trn_boot import _ntff_profile_via_ctypes
            hook = _ntff_profile_via_ctypes('/opt/axon/libaxon_pjrt.so')
            mod = types.ModuleType('antenv.axon_hooks')
            mod.get_axon_ntff_profile_hook = lambda: hook
            sys.modules['antenv.axon_hooks'] = mod
        kwargs = {"trace": True, "tmpdir": tmpdir}
    res = run_bass_kernel_spmd(nc, in_maps, list(range(NC)), **kwargs)
    out = np.concatenate([res.results[r]["out_sl"] for r in range(NC)], axis=0)
    out = out + np.asarray(inputs["b_out"], np.float32)[None, :]
    return out, res.exec_time_ns


def kernel(**inputs) -> np.ndarray:
    out, _ = _kernel_impl(inputs)
    return out
